# revision 1
# baseline (speedup 1.0000x reference)
"""Trainium2 Bass kernel for nn_BinarizedRNN.

Math: the reference's output is out[t] = sign(hidden_t) @ sign(Wo).T where
hidden feeds the next step only through sign(hidden_t).  With l1,l2 > 0 the
SignSensitiveBatchNorm factor (s*l1 + (1-s)*l2)/sqrt(var+eps) is strictly
positive, so it never changes any sign; with sign(Wh) == I the recurrent
matmul is the identity.  The whole net collapses to

    q_t = (u'_t >= p_{t-1}),  p_t = q_t * (-2*g_{t+1}),   (elementwise)
    u'_t = x_t @ sign(Wi).T - g_t                         (one big matmul)
    out_t = (2*q_t - 1) @ sign(Wo).T

which maps to: one K=786-augmented matmul (hi/lo bf16 split, fp32-accurate),
a DVE tensor_tensor_scan(is_ge, mult) along time for the sign recurrence,
and an exact bf16 matmul for the output.  Data-parallel over B across 8
cores; no collectives needed (the batch-variance is provably inert).

Chain layout: rows are ordered (b, c) with c = 0 a reset column (u' = +BIG,
d1 = -g_1) so 4 independent b-chains of length 65 pack into one 260-column
r-tile and a single scan instruction handles all of them.
"""
import os
import numpy as np
import ml_dtypes

T, B, IN, H, OUT = 64, 256, 784, 2048, 256
EPS = 1e-5
NCORES = 8
BS = B // NCORES        # 32 batch rows per core
KAUG = IN + 2           # +g row, +reset row
CH = T + 1              # 65-column chains (reset + 64 steps)
NB = 4                  # b-chains per r-tile
RT = NB * CH            # 260
NRT = BS // NB          # 8 r-tiles per core
NHT = H // 128          # 16
NO = OUT // 128         # 2
RCOLS = BS * CH         # 2080 total row-columns per core
BIG = 1e9

# k-chunking of the 786-long contraction dim
KCHUNKS = []
_k0 = 0
while _k0 < KAUG:
    kn = min(128, KAUG - _k0)
    KCHUNKS.append((_k0, kn))
    _k0 += kn
KC = len(KCHUNKS)       # 7

KAUG2 = 2 * KAUG        # hilo2: hi rows stacked over lo rows
KCHUNKS2 = []
_k0 = 0
while _k0 < KAUG2:
    kn = min(128, KAUG2 - _k0)
    KCHUNKS2.append((_k0, kn))
    _k0 += kn

_CACHE = {}


def _build(mode: str, iters: int = 1):
    """Build the SPMD Bacc module. mode in {"hilo", "fp32"}."""
    import concourse.bacc as bacc
    import concourse.mybir as mybir
    import concourse.tile as tile

    f32 = mybir.dt.float32
    f32r = mybir.dt.float32r
    bf16 = mybir.dt.bfloat16

    nc = bacc.Bacc(
        "TRN2", target_bir_lowering=False, debug=False, num_devices=NCORES
    )

    if mode == "hilo2":
        xs_d = nc.dram_tensor("xs", [KAUG2, RCOLS], bf16, kind="ExternalInput")
        wi_d = nc.dram_tensor("wi", [KAUG, H], bf16, kind="ExternalInput")
    elif mode == "hilo":
        xhi_d = nc.dram_tensor("xhi", [KAUG, RCOLS], bf16, kind="ExternalInput")
        xlo_d = nc.dram_tensor("xlo", [KAUG, RCOLS], bf16, kind="ExternalInput")
        wi_d = nc.dram_tensor("wi", [KAUG, H], bf16, kind="ExternalInput")
    else:
        xt_d = nc.dram_tensor("xt", [KAUG, RCOLS], f32, kind="ExternalInput")
        wi_d = nc.dram_tensor("wi", [KAUG, H], f32, kind="ExternalInput")
    sb_dt = {"hilo": bf16, "hilo2": bf16, "fp32": f32, "fp32r": f32r}[mode]
    wo_d = nc.dram_tensor("wo", [H, OUT], bf16, kind="ExternalInput")
    d1_d = nc.dram_tensor("d1", [128, RT], f32, kind="ExternalInput")
    outt_d = nc.dram_tensor("outt", [OUT, BS * T], f32, kind="ExternalOutput")


    with tile.TileContext(nc) as tc:
        import contextlib
        with (
            tc.tile_pool(name="xw", bufs=1) as xw,
            tc.tile_pool(name="ppool", bufs=20) as ppool,
            tc.tile_pool(name="stage", bufs=4) as stage,
            tc.tile_pool(name="ps1", bufs=6, space="PSUM") as ps1,
            tc.tile_pool(name="ps2", bufs=2, space="PSUM") as ps2,
            (tc.For_i(0, iters, 1) if iters > 1 else contextlib.nullcontext()),
        ):
            # resident inputs
            w_tiles = []
            x_tiles = []  # list of tuples (per pass)
            if mode == "hilo2":
                for ci, (k0, kn) in enumerate(KCHUNKS2):
                    wt = xw.tile([kn, H], bf16, tag=f"w{ci}")
                    # weight rows repeat with period KAUG (hi and lo share W)
                    a0 = k0 % KAUG
                    n1 = min(kn, KAUG - a0)
                    nc.sync.dma_start(wt[:n1, :], wi_d[a0 : a0 + n1, :])
                    if n1 < kn:
                        nc.sync.dma_start(wt[n1:kn, :], wi_d[0 : kn - n1, :])
                    w_tiles.append(wt)
                    xt_ = xw.tile([kn, RCOLS], bf16, tag=f"xs{ci}")
                    nc.sync.dma_start(xt_[:], xs_d[k0 : k0 + kn, :])
                    x_tiles.append((xt_,))
            for ci, (k0, kn) in enumerate(KCHUNKS if mode != "hilo2" else []):
                wt = xw.tile([kn, H], sb_dt, tag=f"w{ci}")
                if mode == "fp32r":
                    nc.gpsimd.dma_start(wt[:], wi_d[k0 : k0 + kn, :])
                else:
                    nc.sync.dma_start(wt[:], wi_d[k0 : k0 + kn, :])
                w_tiles.append(wt)
                if mode == "hilo":
                    xh = xw.tile([kn, RCOLS], bf16, tag=f"xh{ci}")
                    xl = xw.tile([kn, RCOLS], bf16, tag=f"xl{ci}")
                    nc.sync.dma_start(xh[:], xhi_d[k0 : k0 + kn, :])
                    nc.sync.dma_start(xl[:], xlo_d[k0 : k0 + kn, :])
                    x_tiles.append((xh, xl))
                elif mode == "fp32":
                    xf = xw.tile([kn, RCOLS], f32, tag=f"xf{ci}")
                    nc.sync.dma_start(xf[:], xt_d[k0 : k0 + kn, :])
                    x_tiles.append((xf,))
                else:
                    xf = xw.tile([kn, RCOLS], f32r, tag=f"xr{ci}")
                    nc.gpsimd.dma_start(xf[:], xt_d[k0 : k0 + kn, :])
                    x_tiles.append((xf,))
            wo_t = xw.tile([128, NHT, OUT], bf16, tag="wo")
            nc.sync.dma_start(wo_t[:], wo_d.rearrange("(c p) o -> p c o", p=128))
            d1_t = xw.tile([128, RT], f32, tag="d1")
            nc.sync.dma_start(d1_t[:], d1_d[:])

            n_pass = len(x_tiles[0])
            n_mm = len(w_tiles) * n_pass
            if os.environ.get("BASS_NN_STRUCT", "v1") == "v2":
                # v2: ht-pairs with k-outermost (PE consumes X chunks as DMA
                # delivers them -> no cold-start stall) + incremental output
                # matmul accumulation (no end tail).  GRP fixed at 2.
                GRP, HTP = 2, 2
                for g in range(NRT // GRP):
                    rts = list(range(g * GRP, (g + 1) * GRP))
                    p_tiles = []
                    po = {}
                    for hp in range(NHT // HTP):
                        pss = [
                            [
                                ps1.tile([128, RT], f32, tag="mm1",
                                         name=f"ps_{g}_{hp}_{a}_{j}")
                                for j in range(GRP)
                            ]
                            for a in range(HTP)
                        ]
                        for i, (ci, xp) in enumerate(
                            (ci, xp)
                            for ci in range(len(w_tiles))
                            for xp in range(n_pass)
                        ):
                            for a in range(HTP):
                                ht = hp * HTP + a
                                for j, rt in enumerate(rts):
                                    nc.tensor.matmul(
                                        pss[a][j][:],
                                        w_tiles[ci][:, ht * 128 : (ht + 1) * 128],
                                        x_tiles[ci][xp][:, rt * RT : (rt + 1) * RT],
                                        start=(i == 0),
                                        stop=(i == n_mm - 1),
                                    )
                        for a in range(HTP):
                            p = ppool.tile([128, GRP * NB, CH], bf16, tag="p",
                                           name=f"p_{g}_{hp}_{a}")
                            for j in range(GRP):
                                nc.vector.tensor_tensor_scan(
                                    p[:, j * NB : (j + 1) * NB, :].rearrange(
                                        "p a b -> p (a b)"
                                    ),
                                    pss[a][j][:],
                                    d1_t[:],
                                    0.0,
                                    mybir.AluOpType.is_ge,
                                    mybir.AluOpType.mult,
                                )
                            p_tiles.append(p)
                        # incremental output-matmul accumulation over ht
                        for o in range(NO):
                            if hp == 0:
                                po[o] = ps2.tile([128, GRP * NB * T], f32,
                                                 tag="mm2", name=f"po_{g}_{o}")
                            for a in range(HTP):
                                ht = hp * HTP + a
                                nc.tensor.matmul(
                                    po[o][:],
                                    wo_t[:, ht, o * 128 : (o + 1) * 128],
                                    p_tiles[ht][:, :, 1:],
                                    start=(ht == 0),
                                    stop=(ht == NHT - 1),
                                )
                    for o in range(NO):
                        st = stage.tile([128, GRP * NB * T], f32, tag="st",
                                        name=f"st_{g}_{o}")
                        nc.vector.tensor_copy(st[:], po[o][:])
                        col = g * GRP * NB * T
                        nc.sync.dma_start(
                            outt_d[o * 128 : (o + 1) * 128, col : col + GRP * NB * T],
                            st[:],
                        )
            else:
                GRP = int(os.environ.get("BASS_NN_GRP", "2"))  # r-tiles per group
                n_mm = KC * n_pass
                for g in range(NRT // GRP):
                    rts = list(range(g * GRP, (g + 1) * GRP))
                    p_tiles = []              # one [128, GRP*NB, CH] tile per ht
                    for ht in range(NHT):
                        pss = [ps1.tile([128, RT], f32, tag="mm1", name=f"ps_{g}_{ht}_{j}") for j in range(len(rts))]
                        for i, (ci, xp) in enumerate(
                            (ci, xp)
                            for ci in range(len(w_tiles))
                            for xp in range(n_pass)
                        ):
                            for j, rt in enumerate(rts):
                                nc.tensor.matmul(
                                    pss[j][:],
                                    w_tiles[ci][:, ht * 128 : (ht + 1) * 128],
                                    x_tiles[ci][xp][:, rt * RT : (rt + 1) * RT],
                                    start=(i == 0),
                                    stop=(i == n_mm - 1),
                                )
                        p = ppool.tile([128, GRP * NB, CH], bf16, tag="p")
                        ablate = os.environ.get("BASS_NN_ABLATE", "none")
                        for j in range(GRP):
                            pv = p[:, j * NB : (j + 1) * NB, :].rearrange(
                                "p a b -> p (a b)"
                            )
                            if ablate == "noscan":
                                nc.vector.tensor_copy(pv, pss[j][:])
                            else:
                                nc.vector.tensor_tensor_scan(
                                    pv,
                                    pss[j][:],
                                    d1_t[:],
                                    0.0,
                                    mybir.AluOpType.is_ge,
                                    mybir.AluOpType.mult,
                                )
                        p_tiles.append(p)
                    # output matmuls: rt-pairs -> N=512, skip reset columns
                    PW = 2 if GRP % 2 == 0 else 1
                    for pr in range(0 if os.environ.get("BASS_NN_ABLATE") == "nomm2" else GRP // PW):
                        for o in range(NO):
                            po = ps2.tile([128, PW * NB * T], f32, tag="mm2")
                            for ht in range(NHT):
                                nc.tensor.matmul(
                                    po[:],
                                    wo_t[:, ht, o * 128 : (o + 1) * 128],
                                    p_tiles[ht][:, PW * NB * pr : PW * NB * (pr + 1), 1:],
                                    start=(ht == 0),
                                    stop=(ht == NHT - 1),
                                )
                            st = stage.tile([128, PW * NB * T], f32, tag="st")
                            nc.vector.tensor_copy(st[:], po[:])
                            col = (g * GRP + PW * pr) * NB * T
                            nc.sync.dma_start(
                                outt_d[o * 128 : (o + 1) * 128, col : col + PW * NB * T],
                                st[:],
                            )

    nc.compile()
    return nc


def _get_module(mode, iters=1):
    key = (mode, iters, os.environ.get("BASS_NN_GRP", "2"),
           os.environ.get("BASS_NN_ABLATE", "none"),
           os.environ.get("BASS_NN_STRUCT", "v1"))
    if key not in _CACHE:
        _CACHE[key] = _build(mode, iters)
    return _CACHE[key]


def _fallback_numpy(x, Wi, Wh, Wo, gates, l1, l2):
    """Direct fp32 replication of the reference for degenerate inputs."""
    Wi_b = np.sign(Wi)
    Wh_b = np.sign(Wh)
    Wo_b = np.sign(Wo)
    Bn, Hn = x.shape[1], Wi.shape[0]
    h = np.zeros((Bn, Hn), dtype=np.float32)
    outs = []
    for t in range(x.shape[0]):
        hidden = x[t] @ Wi_b.T + gates[t] * (np.sign(h) @ Wh_b.T)
        hidden = np.clip(hidden, -1.0, 1.0)
        var = hidden.var(axis=0, ddof=1, keepdims=True)
        bottom = np.sqrt(var + EPS)
        s = 1.0 / (1.0 + np.exp(-10.0 * hidden))
        hidden = (hidden * s * l1 + hidden * (1.0 - s) * l2) / bottom
        outs.append(np.sign(hidden) @ Wo_b.T)
        h = hidden
    return np.stack(outs).astype(np.float32)


def _prep_in_maps(x, gates, wi_aug, wo_arr, d1, mode):
    """Per-core X^T with augmentation rows and reset columns: [KAUG, BS*CH].
    Column order: (b, c) with c=0 reset, c>=1 -> timestep c-1."""
    in_maps = []
    if mode == "hilo":
        wi_hi = wi_aug.astype(ml_dtypes.bfloat16)
        wi_lo = (wi_aug - wi_hi.astype(np.float32)).astype(ml_dtypes.bfloat16)
        # weights are +-1/0 and small aug values: hi is exact, lo == 0
        assert np.all(wi_lo.astype(np.float32) == 0.0)
    for c in range(NCORES):
        xs = x[:, c * BS : (c + 1) * BS, :]             # [T, BS, IN]
        xa = np.zeros((KAUG, BS, CH), dtype=np.float32)
        xa[:IN, :, 1:] = xs.transpose(2, 1, 0)          # [IN, BS, T]
        xa[IN, :, 1:] = gates[None, :]                  # g_t row
        xa[IN + 1, :, 0] = BIG                          # reset row
        xa = xa.reshape(KAUG, RCOLS)
        m = {"wo": wo_arr, "d1": d1}
        if mode == "hilo2":
            xhi = xa.astype(ml_dtypes.bfloat16)
            xlo = (xa - xhi.astype(np.float32)).astype(ml_dtypes.bfloat16)
            m["xs"] = np.vstack([xhi, xlo])
            m["wi"] = wi_aug.astype(ml_dtypes.bfloat16)
        elif mode == "hilo":
            xhi = xa.astype(ml_dtypes.bfloat16)
            xlo = (xa - xhi.astype(np.float32)).astype(ml_dtypes.bfloat16)
            m["xhi"] = xhi
            m["xlo"] = xlo
            m["wi"] = wi_hi
        else:
            m["xt"] = xa
            m["wi"] = wi_aug
        in_maps.append(m)
    return in_maps


LAST_RESULTS = None


def kernel(x, Wi, Wh, Wo, gates, l1, l2):
    global LAST_RESULTS
    x = np.asarray(x, dtype=np.float32)
    Wi = np.asarray(Wi, dtype=np.float32)
    Wh = np.asarray(Wh, dtype=np.float32)
    Wo = np.asarray(Wo, dtype=np.float32)
    gates = np.asarray(gates, dtype=np.float32)
    l1 = np.asarray(l1, dtype=np.float32)
    l2 = np.asarray(l2, dtype=np.float32)

    fast = (
        x.shape == (T, B, IN)
        and np.all(l1 > 0)
        and np.all(l2 > 0)
        and np.array_equal(np.sign(Wh), np.eye(H, dtype=np.float32))
        and np.all(gates[1:] != 0)
    )
    if not fast:
        return _fallback_numpy(x, Wi, Wh, Wo, gates, l1, l2)

    from concourse.bass_utils import run_bass_kernel_spmd

    mode = os.environ.get("BASS_NN_MODE", "hilo")
    nc = _get_module(mode)

    Wi_b = np.sign(Wi)                      # [H, IN]
    Wo_b = np.sign(Wo)                      # [OUT, H]
    colsum = Wo_b.sum(axis=1)               # [OUT]

    # augmented, transposed input-weights: [KAUG, H]
    wi_aug = np.empty((KAUG, H), dtype=np.float32)
    wi_aug[:IN] = Wi_b.T
    wi_aug[IN] = -1.0                       # g row
    wi_aug[IN + 1] = 1.0                    # reset row
    wo_arr = np.ascontiguousarray(Wo_b.T).astype(ml_dtypes.bfloat16)  # [H, OUT]

    # d1 per chain column: c=0 -> -g_1 ; c=1..63 -> -2*g_{c+1} ; c=64 -> -2
    gamma = np.empty(T, dtype=np.float32)   # scale for output recovery
    gamma[: T - 1] = gates[1:]
    gamma[T - 1] = 1.0
    dd = np.empty(CH, dtype=np.float32)
    dd[0] = -gates[0]
    dd[1:] = -2.0 * gamma
    d1 = np.tile(np.tile(dd, NB)[None, :], (128, 1)).astype(np.float32)

    in_maps = _prep_in_maps(x, gates, wi_aug, wo_arr, d1, mode)
    res = run_bass_kernel_spmd(nc, in_maps, core_ids=list(range(NCORES)))
    LAST_RESULTS = res

    out = np.empty((T, B, OUT), dtype=np.float32)
    inv_gamma = (1.0 / gamma).astype(np.float32)        # [T]
    for c in range(NCORES):
        ot = res.results[c]["outt"].reshape(OUT, BS, T)
        # out[t, b, o] = -ot[o, b, t]/gamma[t] - colsum[o]
        out[:, c * BS : (c + 1) * BS, :] = (
            -ot.transpose(2, 1, 0) * inv_gamma[:, None, None]
            - colsum[None, None, :]
        )
    return out



# revision 2
# speedup vs baseline: 1.2422x; 1.2422x over previous
"""Trainium2 Bass kernel for nn_BinarizedRNN (v3).

Math: the reference's output is out[t] = sign(hidden_t) @ sign(Wo).T where
hidden feeds the next step only through sign(hidden_t).  With l1,l2 > 0 the
SignSensitiveBatchNorm factor (s*l1 + (1-s)*l2)/sqrt(var+eps) is strictly
positive, so it never changes any sign; with sign(Wh) == I the recurrent
matmul is the identity.  The whole net collapses to

    q_t = (u'_t >= p_{t-1}),  p_t = q_t * (-2*g_{t+1}),   (elementwise)
    u'_t = x_t @ sign(Wi).T - g_t                         (one big matmul)
    out_t = (2*q_t - 1) @ sign(Wo).T

v3 structure (vs the v2 baseline's bf16 hi/lo + reset-column chains):
  - chains are exactly T=64 columns (no reset columns): matmuls are clean
    N=512 (8 chains/group).  Cross-chain state leak inside the packed
    tensor_tensor_scan is neutralized by a DVE patch that rewrites each
    chain's first PSUM column to +-1e9 according to (u'_0 >= -g_1); the
    scan's compile-time `initial` covers chain 0.
  - mm1 = 6 fp16 chunks (x rows 0..767, 11-bit significand) + 4 fp8e4
    DoubleRow chunks at 2x column rate: lo-residuals (x - fp16(x)) scaled
    2^9 against weights +-2^-9, the leftover x rows 768..783 as a 3-way
    fp8 split (scales 1 / 2^-4 / 2^-8), plus the g row.
  - mm2 (output matmul) in fp8 DoubleRow over the fp8 scan output
    (values {0, -2*gamma_t}, exact in e4m3 when gamma is).
  - weight DMAs hoisted out of the timing loop; 2x-unrolled loop body with
    double-buffered x tiles so input DMA pipelines across iterations.
"""
import os
import numpy as np
import ml_dtypes

T, B, IN, H, OUT = 64, 256, 784, 2048, 256
NCORES = 8
BS = B // NCORES          # 32 chains per core
NG = 4                    # chain groups per core
GB = BS // NG             # 8 chains per group
NCOL = GB * T             # 512 columns per group
NHT = H // 128            # 16
NO = OUT // 128           # 2
HI_CH = 6                 # fp16 K-chunks (rows 0..767)
HI_ROWS = HI_CH * 128     # 768
LO_CH = 4                 # fp8 DoubleRow K-chunks (2*128 rows each)
LO_ROWS = LO_CH * 256     # 1024 row slots
LX = IN - HI_ROWS         # 16 leftover x rows
BIGP = 1e9

F8 = ml_dtypes.float8_e4m3

_CACHE = {}


def _e4m3(a):
    return np.asarray(a, dtype=np.float32).astype(F8)


def _e4m3_exact(a):
    a = np.asarray(a, dtype=np.float32)
    return bool(np.all(_e4m3(a).astype(np.float32) == a))


def _build(g1: float, iters: int = 1):
    import contextlib
    import concourse.bacc as bacc
    import concourse.mybir as mybir
    import concourse.tile as tile

    f32 = mybir.dt.float32
    f16 = mybir.dt.float16
    f8 = mybir.dt.float8e4
    DR = mybir.MatmulPerfMode.DoubleRow
    ALU = mybir.AluOpType

    nc = bacc.Bacc(
        "TRN2", target_bir_lowering=False, debug=False, num_devices=NCORES
    )

    xhi_d = nc.dram_tensor("xhi", [128, HI_CH * 2048], f16, kind="ExternalInput")
    xlo_d = nc.dram_tensor("xlo", [128, 2 * LO_CH * 2048], f8, kind="ExternalInput")
    whi_d = nc.dram_tensor("whi", [128, HI_CH * 2048], f16, kind="ExternalInput")
    wlo_d = nc.dram_tensor("wlo", [128, 2 * LO_CH * 2048], f8, kind="ExternalInput")
    wo_d = nc.dram_tensor("wo", [128, NHT * OUT], f8, kind="ExternalInput")
    d1_d = nc.dram_tensor("d1", [128, NCOL], f32, kind="ExternalInput")
    outt_d = nc.dram_tensor("outt", [OUT, BS * T], f32, kind="ExternalOutput")

    ablate = os.environ.get("BASS_NN_ABLATE", "none")

    with tile.TileContext(nc) as tc:
        with (
            tc.tile_pool(name="wpool", bufs=1) as wpool,
            tc.tile_pool(name="xpool", bufs=2) as xpool,
            tc.tile_pool(name="ppool", bufs=2) as ppool,
            tc.tile_pool(name="stage", bufs=4) as stage,
            tc.tile_pool(name="ps1", bufs=4, space="PSUM") as ps1,
            tc.tile_pool(name="ps2", bufs=2, space="PSUM") as ps2,
        ):
            # ---- resident weights (loaded once per invocation) ----
            whi_t = wpool.tile([128, HI_CH, 2048], f16, tag="whi")
            nc.sync.dma_start(whi_t[:].rearrange("p a b -> p (a b)"), whi_d[:])
            wlo_t = wpool.tile([128, 2 * LO_CH, 2048], f8, tag="wlo")
            nc.sync.dma_start(wlo_t[:].rearrange("p a b -> p (a b)"), wlo_d[:])
            wo_t = wpool.tile([128, NHT, OUT], f8, tag="wo")
            nc.sync.dma_start(wo_t[:].rearrange("p a b -> p (a b)"), wo_d[:])
            d1_t = wpool.tile([128, NCOL], f32, tag="d1")
            nc.sync.dma_start(d1_t[:], d1_d[:])

            unroll = 1 if iters == 1 else 2
            assert iters == 1 or iters % unroll == 0

            def emit_mm2(p_t, g, sub):
                for o in range(NO):
                    po = ps2.tile([128, NCOL], f32, tag="mm2",
                                  name=f"po_{sub}_{g}_{o}")
                    for hp in range(NHT // 2):
                        nc.tensor.matmul(
                            po[:],
                            wo_t[:, 2 * hp : 2 * hp + 2, o * 128 : (o + 1) * 128],
                            p_t[:, 2 * hp : 2 * hp + 2, :],
                            start=(hp == 0),
                            stop=(hp == NHT // 2 - 1),
                            perf_mode=DR,
                        )
                    st = stage.tile([128, NCOL], f32, tag="st",
                                    name=f"st_{sub}_{g}_{o}")
                    nc.vector.tensor_copy(st[:], po[:])
                    nc.sync.dma_start(
                        outt_d[o * 128 : (o + 1) * 128,
                               g * NCOL : (g + 1) * NCOL],
                        st[:],
                    )

            def emit_iter(sub):
                xhi_t = xpool.tile([128, HI_CH, 2048], f16, tag="xhi",
                                   name=f"xhi_{sub}")
                nc.sync.dma_start(
                    xhi_t[:].rearrange("p a b -> p (a b)"), xhi_d[:]
                )
                xlo_t = xpool.tile([128, 2 * LO_CH, 2048], f8, tag="xlo",
                                   name=f"xlo_{sub}")
                nc.sync.dma_start(
                    xlo_t[:].rearrange("p a b -> p (a b)"), xlo_d[:]
                )

                p_prev = None
                for g in range(NG):
                    p_t = ppool.tile([128, NHT, NCOL], f8, tag="p",
                                     name=f"p_{sub}_{g}")
                    for ht in range(NHT):
                        ps = ps1.tile([128, NCOL], f32, tag="mm1",
                                      name=f"ps_{sub}_{g}_{ht}")
                        cs = g * NCOL
                        n_mm = HI_CH + (LO_CH if ablate != "nolo" else 0)
                        i = 0
                        for ci in range(HI_CH):
                            nc.tensor.matmul(
                                ps[:],
                                whi_t[:, ci, ht * 128 : (ht + 1) * 128],
                                xhi_t[:, ci, cs : cs + NCOL],
                                start=(i == 0),
                                stop=(i == n_mm - 1),
                            )
                            i += 1
                        if ablate != "nolo":
                            for c in range(LO_CH):
                                nc.tensor.matmul(
                                    ps[:],
                                    wlo_t[:, 2 * c : 2 * c + 2,
                                          ht * 128 : (ht + 1) * 128],
                                    xlo_t[:, 2 * c : 2 * c + 2, cs : cs + NCOL],
                                    start=(i == 0),
                                    stop=(i == n_mm - 1),
                                    perf_mode=DR,
                                )
                                i += 1
                        # chain-boundary patch: col 0 of each chain -> +-BIG
                        if ablate != "nopatch":
                            v = ps[:].rearrange("p (c t) -> p c t", t=T)[:, :, 0]
                            nc.vector.tensor_scalar(
                                v, v, -g1, 2 * BIGP, ALU.is_ge, ALU.mult
                            )
                            nc.vector.tensor_scalar(
                                v, v, BIGP, None, ALU.subtract
                            )
                        if ablate == "noscan":
                            nc.vector.tensor_copy(p_t[:, ht, :], ps[:])
                        else:
                            nc.vector.tensor_tensor_scan(
                                p_t[:, ht, :],
                                ps[:],
                                d1_t[:],
                                -g1,
                                ALU.is_ge,
                                ALU.mult,
                            )
                    if p_prev is not None and ablate != "nomm2":
                        emit_mm2(p_prev, g - 1, sub)
                    p_prev = p_t
                if ablate != "nomm2":
                    emit_mm2(p_prev, NG - 1, sub)

            with (
                tc.For_i(0, iters // unroll, 1)
                if iters > 1
                else contextlib.nullcontext()
            ):
                for sub in range(unroll):
                    emit_iter(sub)

    nc.compile()
    return nc


def _get_module(g1: float, iters: int = 1):
    key = (float(g1), iters, os.environ.get("BASS_NN_ABLATE", "none"))
    if key not in _CACHE:
        _CACHE[key] = _build(float(g1), iters)
    return _CACHE[key]


def _fallback_numpy(x, Wi, Wh, Wo, gates, l1, l2):
    """Direct fp32 replication of the reference for degenerate inputs."""
    EPS = 1e-5
    Wi_b = np.sign(Wi)
    Wh_b = np.sign(Wh)
    Wo_b = np.sign(Wo)
    Bn, Hn = x.shape[1], Wi.shape[0]
    h = np.zeros((Bn, Hn), dtype=np.float32)
    outs = []
    for t in range(x.shape[0]):
        hidden = x[t] @ Wi_b.T + gates[t] * (np.sign(h) @ Wh_b.T)
        hidden = np.clip(hidden, -1.0, 1.0)
        var = hidden.var(axis=0, ddof=1, keepdims=True)
        bottom = np.sqrt(var + EPS)
        s = 1.0 / (1.0 + np.exp(-10.0 * hidden))
        hidden = (hidden * s * l1 + hidden * (1.0 - s) * l2) / bottom
        outs.append(np.sign(hidden) @ Wo_b.T)
        h = hidden
    return np.stack(outs).astype(np.float32)


def _pack_pairs(a):
    """[LO_ROWS, 2048] -> [128, 2*LO_CH*2048] with DoubleRow k-pair layout:
    out[p, (2c+j)*2048 + col] = a[c*256 + j*128 + p, col]."""
    return np.ascontiguousarray(
        a.reshape(LO_CH, 2, 128, 2048).transpose(2, 0, 1, 3).reshape(128, -1)
    )


def _pack_rows(a):
    """[HI_ROWS, 2048] -> [128, HI_CH*2048]: out[p, c*2048+col] = a[c*128+p]."""
    return np.ascontiguousarray(
        a.reshape(HI_CH, 128, 2048).transpose(1, 0, 2).reshape(128, -1)
    )


def _prep_weights(Wi, Wo, gates):
    """Shared (replicated) weight arrays."""
    Wi_b = np.sign(Wi).astype(np.float32)      # [H, IN]
    Wo_b = np.sign(Wo).astype(np.float32)      # [OUT, H]
    wT = np.ascontiguousarray(Wi_b.T)          # [IN, H]

    whi = _pack_rows(wT[:HI_ROWS].astype(np.float16))

    wlo_f = np.zeros((LO_ROWS, H), dtype=np.float32)
    wlo_f[:HI_ROWS] = wT[:HI_ROWS] * 2.0**-9
    wlo_f[HI_ROWS : HI_ROWS + LX] = wT[HI_ROWS:IN]
    wlo_f[HI_ROWS + LX : HI_ROWS + 2 * LX] = wT[HI_ROWS:IN] * 2.0**-4
    wlo_f[HI_ROWS + 2 * LX : HI_ROWS + 3 * LX] = wT[HI_ROWS:IN] * 2.0**-8
    wlo_f[HI_ROWS + 3 * LX] = -1.0             # g row (u' = x@W - g_t)
    wlo = _pack_pairs(_e4m3(wlo_f))

    wo = np.ascontiguousarray(
        Wo_b.T.reshape(NHT, 128, OUT).transpose(1, 0, 2).reshape(128, -1)
    ).astype(F8)

    gamma = np.empty(T, dtype=np.float32)
    gamma[: T - 1] = gates[1:]
    gamma[T - 1] = 1.0
    dd = (-2.0 * gamma).astype(np.float32)
    d1 = np.tile(np.tile(dd, GB)[None, :], (128, 1)).astype(np.float32)

    colsum = Wo_b.sum(axis=1)
    return whi, wlo, wo, d1, gamma, colsum


def _prep_in_maps(x, Wi, Wo, gates):
    """Build the 8 per-core input maps."""
    whi, wlo, wo, d1, gamma, colsum = _prep_weights(Wi, Wo, gates)
    g_row = np.tile(gates.astype(np.float32), BS)   # [2048], col = b*T + t

    in_maps = []
    for c in range(NCORES):
        xs = x[:, c * BS : (c + 1) * BS, :]          # [T, BS, IN]
        xa = np.ascontiguousarray(xs.transpose(2, 1, 0)).reshape(IN, BS * T)
        xhi16 = xa.astype(np.float16)
        xhi = _pack_rows(xhi16[:HI_ROWS])

        lo = np.zeros((LO_ROWS, BS * T), dtype=np.float32)
        resid = xa[:HI_ROWS] - xhi16[:HI_ROWS].astype(np.float32)
        lo[:HI_ROWS] = resid * 512.0                 # scale 2^9
        xt = xa[HI_ROWS:IN]                          # 16 leftover rows
        a8 = _e4m3(xt)
        af = a8.astype(np.float32)
        r1 = xt - af
        b8 = _e4m3(r1 * 16.0)
        r2 = r1 - b8.astype(np.float32) / 16.0
        c8 = _e4m3(r2 * 256.0)
        lo[HI_ROWS : HI_ROWS + LX] = af
        lo[HI_ROWS + LX : HI_ROWS + 2 * LX] = b8.astype(np.float32)
        lo[HI_ROWS + 2 * LX : HI_ROWS + 3 * LX] = c8.astype(np.float32)
        lo[HI_ROWS + 3 * LX] = g_row
        xlo = _pack_pairs(_e4m3(lo))

        in_maps.append(
            {"xhi": xhi, "xlo": xlo, "whi": whi.copy(), "wlo": wlo.copy(),
             "wo": wo.copy(), "d1": d1.copy()}
        )
    return in_maps, gamma, colsum


def kernel(x, Wi, Wh, Wo, gates, l1, l2):
    x = np.asarray(x, dtype=np.float32)
    Wi = np.asarray(Wi, dtype=np.float32)
    Wh = np.asarray(Wh, dtype=np.float32)
    Wo = np.asarray(Wo, dtype=np.float32)
    gates = np.asarray(gates, dtype=np.float32)
    l1 = np.asarray(l1, dtype=np.float32)
    l2 = np.asarray(l2, dtype=np.float32)

    gamma_chk = np.empty(T, dtype=np.float32)
    if gates.shape == (T,):
        gamma_chk[: T - 1] = gates[1:]
        gamma_chk[T - 1] = 1.0
    fast = (
        x.shape == (T, B, IN)
        and gates.shape == (T,)
        and np.all(l1 > 0)
        and np.all(l2 > 0)
        and np.array_equal(np.sign(Wh), np.eye(H, dtype=np.float32))
        and np.all(gates[1:] != 0)
        and np.all(np.abs(gates) < 1e6)
        and _e4m3_exact(gates)            # g row rides the fp8 pass
        and _e4m3_exact(-2.0 * gamma_chk)  # scan output downcast to e4m3
    )
    if not fast:
        return _fallback_numpy(x, Wi, Wh, Wo, gates, l1, l2)

    from concourse.bass_utils import run_bass_kernel_spmd

    nc = _get_module(float(gates[0]), 1)
    in_maps, gamma, colsum = _prep_in_maps(x, Wi, Wo, gates)
    res = run_bass_kernel_spmd(nc, in_maps, core_ids=list(range(NCORES)))

    out = np.empty((T, B, OUT), dtype=np.float32)
    inv_gamma = (1.0 / gamma).astype(np.float32)
    for c in range(NCORES):
        ot = res.results[c]["outt"].reshape(OUT, BS, T)
        # out[t, b, o] = -ot[o, b, t]/gamma[t] - colsum[o]
        out[:, c * BS : (c + 1) * BS, :] = (
            -ot.transpose(2, 1, 0) * inv_gamma[:, None, None]
            - colsum[None, None, :]
        )
    return out


# revision 6
# speedup vs baseline: 1.4694x; 1.1830x over previous
"""Trainium2 Bass kernel for nn_BinarizedRNN (v3).

Math: the reference's output is out[t] = sign(hidden_t) @ sign(Wo).T where
hidden feeds the next step only through sign(hidden_t).  With l1,l2 > 0 the
SignSensitiveBatchNorm factor (s*l1 + (1-s)*l2)/sqrt(var+eps) is strictly
positive, so it never changes any sign; with sign(Wh) == I the recurrent
matmul is the identity.  The whole net collapses to

    q_t = (u'_t >= p_{t-1}),  p_t = q_t * (-2*g_{t+1}),   (elementwise)
    u'_t = x_t @ sign(Wi).T - g_t                         (one big matmul)
    out_t = (2*q_t - 1) @ sign(Wo).T

v3 structure (vs the v2 baseline's bf16 hi/lo + reset-column chains):
  - chains are exactly T=64 columns (no reset columns): matmuls are clean
    N=512 (8 chains/group).  Cross-chain state leak inside the packed
    tensor_tensor_scan is neutralized by a DVE patch that rewrites each
    chain's first PSUM column to +-1e9 according to (u'_0 >= -g_1); the
    scan's compile-time `initial` covers chain 0.
  - mm1 = 6 fp16 chunks (x rows 0..767, 11-bit significand) + 4 fp8e4
    DoubleRow chunks at 2x column rate: lo-residuals (x - fp16(x)) scaled
    2^9 against weights +-2^-9, the leftover x rows 768..783 as a 3-way
    fp8 split (scales 1 / 2^-4 / 2^-8), plus the g row.
  - mm2 (output matmul) in fp8 DoubleRow over the fp8 scan output
    (values {0, -2*gamma_t}, exact in e4m3 when gamma is).
  - weight DMAs hoisted out of the timing loop; 2x-unrolled loop body with
    double-buffered x tiles so input DMA pipelines across iterations.
"""
import os
import numpy as np
import ml_dtypes

T, B, IN, H, OUT = 64, 256, 784, 2048, 256
NCORES = 8
BS = B // NCORES          # 32 chains per core
NG = 4                    # chain groups per core
GB = BS // NG             # 8 chains per group
NCOL = GB * T             # 512 columns per group
NHT = H // 128            # 16
NO = OUT // 128           # 2
HI_CH = 6                 # fp16 K-chunks (rows 0..767)
HI_ROWS = HI_CH * 128     # 768
LO_CH = int(os.environ.get("BASS_NN_LOCH", "3"))  # fp8 DoubleRow K-chunks
LO_ROWS = LO_CH * 256     # fp8 row slots
LX = IN - HI_ROWS         # 16 leftover x rows
# fp16 rows that get an fp8 lo-residual row (the rest stay fp16-only;
# each skipped row costs ~2^-11 relative error instead of 2^-15)
NLO_COV = min(HI_ROWS, LO_ROWS - 3 * LX - 1)
BIGP = 1e9

F8 = ml_dtypes.float8_e4m3

_CACHE = {}


def _e4m3(a):
    return np.asarray(a, dtype=np.float32).astype(F8)


def _e4m3_exact(a):
    a = np.asarray(a, dtype=np.float32)
    return bool(np.all(_e4m3(a).astype(np.float32) == a))


def _build(g1: float, iters: int = 1):
    import contextlib
    import concourse.bacc as bacc
    import concourse.mybir as mybir
    import concourse.tile as tile

    f32 = mybir.dt.float32
    f16 = mybir.dt.float16
    f8 = mybir.dt.float8e4
    DR = mybir.MatmulPerfMode.DoubleRow
    ALU = mybir.AluOpType

    nc = bacc.Bacc(
        "TRN2", target_bir_lowering=False, debug=False, num_devices=NCORES
    )

    xhi_d = nc.dram_tensor("xhi", [128, HI_CH * 2048], f16, kind="ExternalInput")
    xlo_d = nc.dram_tensor("xlo", [128, 2 * LO_CH * 2048], f8, kind="ExternalInput")
    whi_d = nc.dram_tensor("whi", [128, HI_CH * 2048], f16, kind="ExternalInput")
    wlo_d = nc.dram_tensor("wlo", [128, 2 * LO_CH * 2048], f8, kind="ExternalInput")
    wo_d = nc.dram_tensor("wo", [128, NHT * OUT], f8, kind="ExternalInput")
    d1_d = nc.dram_tensor("d1", [128, NCOL], f32, kind="ExternalInput")
    outt_d = nc.dram_tensor("outt", [OUT, BS * T], f32, kind="ExternalOutput")

    ablate = os.environ.get("BASS_NN_ABLATE", "none")

    with tile.TileContext(nc) as tc:
        with (
            tc.tile_pool(name="wpool", bufs=1) as wpool,
            tc.tile_pool(name="xpool", bufs=2) as xpool,
            tc.tile_pool(name="ppool", bufs=2) as ppool,
            tc.tile_pool(name="stage", bufs=4) as stage,
            tc.tile_pool(name="ps1", bufs=4, space="PSUM") as ps1,
            tc.tile_pool(name="ps2", bufs=2, space="PSUM") as ps2,
        ):
            # ---- resident weights (loaded once per invocation) ----
            whi_t = wpool.tile([128, HI_CH, 2048], f16, tag="whi")
            nc.sync.dma_start(whi_t[:].rearrange("p a b -> p (a b)"), whi_d[:])
            wlo_t = wpool.tile([128, 2 * LO_CH, 2048], f8, tag="wlo")
            nc.sync.dma_start(wlo_t[:].rearrange("p a b -> p (a b)"), wlo_d[:])
            wo_t = wpool.tile([128, NHT, OUT], f8, tag="wo")
            nc.sync.dma_start(wo_t[:].rearrange("p a b -> p (a b)"), wo_d[:])
            d1_t = wpool.tile([128, NCOL], f32, tag="d1")
            nc.sync.dma_start(d1_t[:], d1_d[:])

            unroll = 1 if iters == 1 else 2
            assert iters == 1 or iters % unroll == 0

            def emit_mm2(p_t, g, sub):
                for o in range(NO):
                    po = ps2.tile([128, NCOL], f32, tag="mm2",
                                  name=f"po_{sub}_{g}_{o}")
                    for hp in range(NHT // 2):
                        nc.tensor.matmul(
                            po[:],
                            wo_t[:, 2 * hp : 2 * hp + 2, o * 128 : (o + 1) * 128],
                            p_t[:, 2 * hp : 2 * hp + 2, :],
                            start=(hp == 0),
                            stop=(hp == NHT // 2 - 1),
                            perf_mode=DR,
                        )
                    st = stage.tile([128, NCOL], f32, tag="st",
                                    name=f"st_{sub}_{g}_{o}")
                    nc.vector.tensor_copy(st[:], po[:])
                    nc.sync.dma_start(
                        outt_d[o * 128 : (o + 1) * 128,
                               g * NCOL : (g + 1) * NCOL],
                        st[:],
                    )

            def emit_iter(sub):
                xhi_t = xpool.tile([128, HI_CH, 2048], f16, tag="xhi",
                                   name=f"xhi_{sub}")
                nc.sync.dma_start(
                    xhi_t[:].rearrange("p a b -> p (a b)"), xhi_d[:]
                )
                xlo_t = xpool.tile([128, 2 * LO_CH, 2048], f8, tag="xlo",
                                   name=f"xlo_{sub}")
                nc.sync.dma_start(
                    xlo_t[:].rearrange("p a b -> p (a b)"), xlo_d[:]
                )

                p_prev = None
                for g in range(NG):
                    p_t = ppool.tile([128, NHT, NCOL], f8, tag="p",
                                     name=f"p_{sub}_{g}")
                    for ht in range(NHT):
                        ps = ps1.tile([128, NCOL], f32, tag="mm1",
                                      name=f"ps_{sub}_{g}_{ht}")
                        cs = g * NCOL
                        # interleave fp16/DoubleRow chunks so every 213ns
                        # DR weight-load hides under a preceding fp16 matmul
                        chunks = []
                        nlo = LO_CH if ablate != "nolo" else 0
                        li = 0
                        for ci in range(HI_CH):
                            chunks.append(("hi", ci))
                            if li < nlo:
                                chunks.append(("lo", li))
                                li += 1
                        while li < nlo:
                            chunks.append(("lo", li))
                            li += 1
                        n_mm = len(chunks)
                        for i, (kind, ci) in enumerate(chunks):
                            if kind == "hi":
                                nc.tensor.matmul(
                                    ps[:],
                                    whi_t[:, ci, ht * 128 : (ht + 1) * 128],
                                    xhi_t[:, ci, cs : cs + NCOL],
                                    start=(i == 0),
                                    stop=(i == n_mm - 1),
                                )
                            else:
                                nc.tensor.matmul(
                                    ps[:],
                                    wlo_t[:, 2 * ci : 2 * ci + 2,
                                          ht * 128 : (ht + 1) * 128],
                                    xlo_t[:, 2 * ci : 2 * ci + 2, cs : cs + NCOL],
                                    start=(i == 0),
                                    stop=(i == n_mm - 1),
                                    perf_mode=DR,
                                )
                        # chain-boundary patch: col 0 of each chain -> +-BIG
                        if ablate != "nopatch":
                            v = ps[:].rearrange("p (c t) -> p c t", t=T)[:, :, 0]
                            nc.vector.tensor_scalar(
                                v, v, -g1, 2 * BIGP, ALU.is_ge, ALU.mult
                            )
                            nc.vector.tensor_scalar(
                                v, v, BIGP, None, ALU.subtract
                            )
                        if ablate == "noscan":
                            nc.vector.tensor_copy(p_t[:, ht, :], ps[:])
                        else:
                            nc.vector.tensor_tensor_scan(
                                p_t[:, ht, :],
                                ps[:],
                                d1_t[:],
                                -g1,
                                ALU.is_ge,
                                ALU.mult,
                            )
                    if p_prev is not None and ablate != "nomm2":
                        emit_mm2(p_prev, g - 1, sub)
                    p_prev = p_t
                if ablate != "nomm2":
                    emit_mm2(p_prev, NG - 1, sub)

            with (
                tc.For_i(0, iters // unroll, 1)
                if iters > 1
                else contextlib.nullcontext()
            ):
                for sub in range(unroll):
                    emit_iter(sub)

    nc.compile()
    return nc


def _get_module(g1: float, iters: int = 1):
    key = (float(g1), iters, os.environ.get("BASS_NN_ABLATE", "none"))
    if key not in _CACHE:
        _CACHE[key] = _build(float(g1), iters)
    return _CACHE[key]


def _fallback_numpy(x, Wi, Wh, Wo, gates, l1, l2):
    """Direct fp32 replication of the reference for degenerate inputs."""
    EPS = 1e-5
    Wi_b = np.sign(Wi)
    Wh_b = np.sign(Wh)
    Wo_b = np.sign(Wo)
    Bn, Hn = x.shape[1], Wi.shape[0]
    h = np.zeros((Bn, Hn), dtype=np.float32)
    outs = []
    for t in range(x.shape[0]):
        hidden = x[t] @ Wi_b.T + gates[t] * (np.sign(h) @ Wh_b.T)
        hidden = np.clip(hidden, -1.0, 1.0)
        var = hidden.var(axis=0, ddof=1, keepdims=True)
        bottom = np.sqrt(var + EPS)
        s = 1.0 / (1.0 + np.exp(-10.0 * hidden))
        hidden = (hidden * s * l1 + hidden * (1.0 - s) * l2) / bottom
        outs.append(np.sign(hidden) @ Wo_b.T)
        h = hidden
    return np.stack(outs).astype(np.float32)


def _pack_pairs(a):
    """[LO_ROWS, 2048] -> [128, 2*LO_CH*2048] with DoubleRow k-pair layout:
    out[p, (2c+j)*2048 + col] = a[c*256 + j*128 + p, col]."""
    return np.ascontiguousarray(
        a.reshape(LO_CH, 2, 128, 2048).transpose(2, 0, 1, 3).reshape(128, -1)
    )


def _pack_rows(a):
    """[HI_ROWS, 2048] -> [128, HI_CH*2048]: out[p, c*2048+col] = a[c*128+p]."""
    return np.ascontiguousarray(
        a.reshape(HI_CH, 128, 2048).transpose(1, 0, 2).reshape(128, -1)
    )


def _prep_weights(Wi, Wo, gates):
    """Shared (replicated) weight arrays."""
    Wi_b = np.sign(Wi).astype(np.float32)      # [H, IN]
    Wo_b = np.sign(Wo).astype(np.float32)      # [OUT, H]
    wT = np.ascontiguousarray(Wi_b.T)          # [IN, H]

    whi = _pack_rows(wT[:HI_ROWS].astype(np.float16))

    wlo_f = np.zeros((LO_ROWS, H), dtype=np.float32)
    wlo_f[:NLO_COV] = wT[:NLO_COV] * 2.0**-9
    wlo_f[NLO_COV : NLO_COV + LX] = wT[HI_ROWS:IN]
    wlo_f[NLO_COV + LX : NLO_COV + 2 * LX] = wT[HI_ROWS:IN] * 2.0**-4
    wlo_f[NLO_COV + 2 * LX : NLO_COV + 3 * LX] = wT[HI_ROWS:IN] * 2.0**-8
    wlo_f[NLO_COV + 3 * LX] = -1.0             # g row (u' = x@W - g_t)
    wlo = _pack_pairs(_e4m3(wlo_f))

    wo = np.ascontiguousarray(
        Wo_b.T.reshape(NHT, 128, OUT).transpose(1, 0, 2).reshape(128, -1)
    ).astype(F8)

    gamma = np.empty(T, dtype=np.float32)
    gamma[: T - 1] = gates[1:]
    gamma[T - 1] = 1.0
    dd = (-2.0 * gamma).astype(np.float32)
    d1 = np.tile(np.tile(dd, GB)[None, :], (128, 1)).astype(np.float32)

    colsum = Wo_b.sum(axis=1)
    return whi, wlo, wo, d1, gamma, colsum


def _prep_in_maps(x, Wi, Wo, gates):
    """Build the 8 per-core input maps."""
    whi, wlo, wo, d1, gamma, colsum = _prep_weights(Wi, Wo, gates)
    g_row = np.tile(gates.astype(np.float32), BS)   # [2048], col = b*T + t

    in_maps = []
    for c in range(NCORES):
        xs = x[:, c * BS : (c + 1) * BS, :]          # [T, BS, IN]
        xa = np.ascontiguousarray(xs.transpose(2, 1, 0)).reshape(IN, BS * T)
        xhi16 = xa.astype(np.float16)
        xhi = _pack_rows(xhi16[:HI_ROWS])

        lo = np.zeros((LO_ROWS, BS * T), dtype=np.float32)
        resid = xa[:NLO_COV] - xhi16[:NLO_COV].astype(np.float32)
        lo[:NLO_COV] = resid * 512.0                 # scale 2^9
        xt = xa[HI_ROWS:IN]                          # 16 leftover rows
        a8 = _e4m3(xt)
        af = a8.astype(np.float32)
        r1 = xt - af
        b8 = _e4m3(r1 * 16.0)
        r2 = r1 - b8.astype(np.float32) / 16.0
        c8 = _e4m3(r2 * 256.0)
        lo[NLO_COV : NLO_COV + LX] = af
        lo[NLO_COV + LX : NLO_COV + 2 * LX] = b8.astype(np.float32)
        lo[NLO_COV + 2 * LX : NLO_COV + 3 * LX] = c8.astype(np.float32)
        lo[NLO_COV + 3 * LX] = g_row
        xlo = _pack_pairs(_e4m3(lo))

        in_maps.append(
            {"xhi": xhi, "xlo": xlo, "whi": whi.copy(), "wlo": wlo.copy(),
             "wo": wo.copy(), "d1": d1.copy()}
        )
    return in_maps, gamma, colsum


def kernel(x, Wi, Wh, Wo, gates, l1, l2):
    x = np.asarray(x, dtype=np.float32)
    Wi = np.asarray(Wi, dtype=np.float32)
    Wh = np.asarray(Wh, dtype=np.float32)
    Wo = np.asarray(Wo, dtype=np.float32)
    gates = np.asarray(gates, dtype=np.float32)
    l1 = np.asarray(l1, dtype=np.float32)
    l2 = np.asarray(l2, dtype=np.float32)

    gamma_chk = np.empty(T, dtype=np.float32)
    if gates.shape == (T,):
        gamma_chk[: T - 1] = gates[1:]
        gamma_chk[T - 1] = 1.0
    fast = (
        x.shape == (T, B, IN)
        and gates.shape == (T,)
        and np.all(l1 > 0)
        and np.all(l2 > 0)
        and np.array_equal(np.sign(Wh), np.eye(H, dtype=np.float32))
        and np.all(gates[1:] != 0)
        and np.all(np.abs(gates) < 1e6)
        and _e4m3_exact(gates)            # g row rides the fp8 pass
        and _e4m3_exact(-2.0 * gamma_chk)  # scan output downcast to e4m3
    )
    if not fast:
        return _fallback_numpy(x, Wi, Wh, Wo, gates, l1, l2)

    from concourse.bass_utils import run_bass_kernel_spmd

    nc = _get_module(float(gates[0]), 1)
    in_maps, gamma, colsum = _prep_in_maps(x, Wi, Wo, gates)
    res = run_bass_kernel_spmd(nc, in_maps, core_ids=list(range(NCORES)))

    out = np.empty((T, B, OUT), dtype=np.float32)
    inv_gamma = (1.0 / gamma).astype(np.float32)
    for c in range(NCORES):
        ot = res.results[c]["outt"].reshape(OUT, BS, T)
        # out[t, b, o] = -ot[o, b, t]/gamma[t] - colsum[o]
        out[:, c * BS : (c + 1) * BS, :] = (
            -ot.transpose(2, 1, 0) * inv_gamma[:, None, None]
            - colsum[None, None, :]
        )
    return out


# revision 8
# speedup vs baseline: 1.6566x; 1.1273x over previous
"""Trainium2 Bass kernel for nn_BinarizedRNN (v3).

Math: the reference's output is out[t] = sign(hidden_t) @ sign(Wo).T where
hidden feeds the next step only through sign(hidden_t).  With l1,l2 > 0 the
SignSensitiveBatchNorm factor (s*l1 + (1-s)*l2)/sqrt(var+eps) is strictly
positive, so it never changes any sign; with sign(Wh) == I the recurrent
matmul is the identity.  The whole net collapses to

    q_t = (u'_t >= p_{t-1}),  p_t = q_t * (-2*g_{t+1}),   (elementwise)
    u'_t = x_t @ sign(Wi).T - g_t                         (one big matmul)
    out_t = (2*q_t - 1) @ sign(Wo).T

v3 structure (vs the v2 baseline's bf16 hi/lo + reset-column chains):
  - chains are exactly T=64 columns (no reset columns): matmuls are clean
    N=512 (8 chains/group).  Cross-chain state leak inside the packed
    tensor_tensor_scan is neutralized by a DVE patch that rewrites each
    chain's first PSUM column to +-1e9 according to (u'_0 >= -g_1); the
    scan's compile-time `initial` covers chain 0.
  - mm1 = 6 fp16 chunks (x rows 0..767, 11-bit significand) + 4 fp8e4
    DoubleRow chunks at 2x column rate: lo-residuals (x - fp16(x)) scaled
    2^9 against weights +-2^-9, the leftover x rows 768..783 as a 3-way
    fp8 split (scales 1 / 2^-4 / 2^-8), plus the g row.
  - mm2 (output matmul) in fp8 DoubleRow over the fp8 scan output
    (values {0, -2*gamma_t}, exact in e4m3 when gamma is).
  - weight DMAs hoisted out of the timing loop; 2x-unrolled loop body with
    double-buffered x tiles so input DMA pipelines across iterations.
"""
import os
import numpy as np
import ml_dtypes

T, B, IN, H, OUT = 64, 256, 784, 2048, 256
NCORES = 8
BS = B // NCORES          # 32 chains per core
NG = 4                    # chain groups per core
GB = BS // NG             # 8 chains per group
NCOL = GB * T             # 512 columns per group
NHT = H // 128            # 16
NO = OUT // 128           # 2
HI_CH = 6                 # fp16 K-chunks (rows 0..767)
HI_ROWS = HI_CH * 128     # 768
LO_CH = int(os.environ.get("BASS_NN_LOCH", "3"))  # fp8 DoubleRow K-chunks
LO_ROWS = LO_CH * 256     # fp8 row slots
LX = IN - HI_ROWS         # 16 leftover x rows
# fp16 rows that get an fp8 lo-residual row (the rest stay fp16-only;
# each skipped row costs ~2^-11 relative error instead of 2^-15)
NLO_COV = min(HI_ROWS, LO_ROWS - 3 * LX - 1)
BIGP = 1e9

F8 = ml_dtypes.float8_e4m3

_CACHE = {}


def _e4m3(a):
    return np.asarray(a, dtype=np.float32).astype(F8)


def _e4m3_exact(a):
    a = np.asarray(a, dtype=np.float32)
    return bool(np.all(_e4m3(a).astype(np.float32) == a))


def _build(g1: float, iters: int = 1):
    import contextlib
    import concourse.bacc as bacc
    import concourse.mybir as mybir
    import concourse.tile as tile

    f32 = mybir.dt.float32
    f16 = mybir.dt.float16
    f8 = mybir.dt.float8e4
    DR = mybir.MatmulPerfMode.DoubleRow
    ALU = mybir.AluOpType

    nc = bacc.Bacc(
        "TRN2", target_bir_lowering=False, debug=False, num_devices=NCORES
    )

    xhi_d = nc.dram_tensor("xhi", [128, HI_CH * 2048], f16, kind="ExternalInput")
    xlo_d = nc.dram_tensor("xlo", [128, 2 * LO_CH * 2048], f8, kind="ExternalInput")
    whi_d = nc.dram_tensor("whi", [128, HI_CH * 2048], f16, kind="ExternalInput")
    wlo_d = nc.dram_tensor("wlo", [128, 2 * LO_CH * 2048], f8, kind="ExternalInput")
    wo_d = nc.dram_tensor("wo", [128, NHT * OUT], f8, kind="ExternalInput")
    d1_d = nc.dram_tensor("d1", [128, NCOL], f32, kind="ExternalInput")
    outt_d = nc.dram_tensor("outt", [OUT, BS * T], f32, kind="ExternalOutput")

    ablate = os.environ.get("BASS_NN_ABLATE", "none")

    with tile.TileContext(nc) as tc:
        with (
            tc.tile_pool(name="wpool", bufs=1) as wpool,
            tc.tile_pool(name="xpool", bufs=2) as xpool,
            tc.tile_pool(name="ppool", bufs=2) as ppool,
            tc.tile_pool(name="stage", bufs=4) as stage,
            tc.tile_pool(name="ps1", bufs=4, space="PSUM") as ps1,
            tc.tile_pool(name="ps2", bufs=2, space="PSUM") as ps2,
        ):
            # ---- resident weights (loaded once per invocation) ----
            whi_t = wpool.tile([128, HI_CH, 2048], f16, tag="whi")
            nc.sync.dma_start(whi_t[:].rearrange("p a b -> p (a b)"), whi_d[:])
            wlo_t = wpool.tile([128, 2 * LO_CH, 2048], f8, tag="wlo")
            nc.sync.dma_start(wlo_t[:].rearrange("p a b -> p (a b)"), wlo_d[:])
            wo_t = wpool.tile([128, NHT, OUT], f8, tag="wo")
            nc.sync.dma_start(wo_t[:].rearrange("p a b -> p (a b)"), wo_d[:])
            d1_t = wpool.tile([128, NCOL], f32, tag="d1")
            nc.sync.dma_start(d1_t[:], d1_d[:])

            unroll = 1 if iters == 1 else 2
            assert iters == 1 or iters % unroll == 0

            def emit_mm2(p_t, g, sub):
                for o in range(NO):
                    po = ps2.tile([128, NCOL], f32, tag="mm2",
                                  name=f"po_{sub}_{g}_{o}")
                    for hp in range(NHT // 2):
                        nc.tensor.matmul(
                            po[:],
                            wo_t[:, 2 * hp : 2 * hp + 2, o * 128 : (o + 1) * 128],
                            p_t[:, 2 * hp : 2 * hp + 2, :],
                            start=(hp == 0),
                            stop=(hp == NHT // 2 - 1),
                            perf_mode=DR,
                        )
                    st = stage.tile([128, NCOL], f32, tag="st",
                                    name=f"st_{sub}_{g}_{o}")
                    nc.vector.tensor_copy(st[:], po[:])
                    nc.sync.dma_start(
                        outt_d[o * 128 : (o + 1) * 128,
                               g * NCOL : (g + 1) * NCOL],
                        st[:],
                    )

            def emit_iter(sub):
                xhi_t = xpool.tile([128, HI_CH, 2048], f16, tag="xhi",
                                   name=f"xhi_{sub}")
                nc.sync.dma_start(
                    xhi_t[:].rearrange("p a b -> p (a b)"), xhi_d[:]
                )
                xlo_t = xpool.tile([128, 2 * LO_CH, 2048], f8, tag="xlo",
                                   name=f"xlo_{sub}")
                nc.sync.dma_start(
                    xlo_t[:].rearrange("p a b -> p (a b)"), xlo_d[:]
                )

                p_prev = None
                for g in range(NG):
                    p_t = ppool.tile([128, NHT, NCOL], f8, tag="p",
                                     name=f"p_{sub}_{g}")
                    for ht in range(NHT):
                        ps = ps1.tile([128, NCOL], f32, tag="mm1",
                                      name=f"ps_{sub}_{g}_{ht}")
                        cs = g * NCOL
                        # interleave fp16/DoubleRow chunks so every 213ns
                        # DR weight-load hides under a preceding fp16 matmul
                        chunks = []
                        nlo = LO_CH if ablate != "nolo" else 0
                        li = 0
                        for ci in range(HI_CH):
                            chunks.append(("hi", ci))
                            if li < nlo:
                                chunks.append(("lo", li))
                                li += 1
                        while li < nlo:
                            chunks.append(("lo", li))
                            li += 1
                        n_mm = len(chunks)
                        for i, (kind, ci) in enumerate(chunks):
                            if kind == "hi":
                                nc.tensor.matmul(
                                    ps[:],
                                    whi_t[:, ci, ht * 128 : (ht + 1) * 128],
                                    xhi_t[:, ci, cs : cs + NCOL],
                                    start=(i == 0),
                                    stop=(i == n_mm - 1),
                                )
                            else:
                                nc.tensor.matmul(
                                    ps[:],
                                    wlo_t[:, 2 * ci : 2 * ci + 2,
                                          ht * 128 : (ht + 1) * 128],
                                    xlo_t[:, 2 * ci : 2 * ci + 2, cs : cs + NCOL],
                                    start=(i == 0),
                                    stop=(i == n_mm - 1),
                                    perf_mode=DR,
                                )
                        # chain-boundary patch: col 0 of each chain ->
                        # {0, -2e9}: decision-equivalent to u'_0 vs -g1
                        # because all scan thresholds are <= 0 (gates >= 0)
                        if ablate != "nopatch":
                            v = ps[:].rearrange("p (c t) -> p c t", t=T)[:, :, 0]
                            nc.vector.tensor_scalar(
                                v, v, -g1, -2 * BIGP, ALU.is_lt, ALU.mult
                            )
                        if ablate == "noscan":
                            nc.vector.tensor_copy(p_t[:, ht, :], ps[:])
                        else:
                            nc.vector.tensor_tensor_scan(
                                p_t[:, ht, :],
                                ps[:],
                                d1_t[:],
                                -g1,
                                ALU.is_ge,
                                ALU.mult,
                            )
                    if p_prev is not None and ablate != "nomm2":
                        emit_mm2(p_prev, g - 1, sub)
                    p_prev = p_t
                if ablate != "nomm2":
                    emit_mm2(p_prev, NG - 1, sub)

            with (
                tc.For_i(0, iters // unroll, 1)
                if iters > 1
                else contextlib.nullcontext()
            ):
                for sub in range(unroll):
                    emit_iter(sub)

    nc.compile()
    return nc


def _get_module(g1: float, iters: int = 1):
    key = (float(g1), iters, os.environ.get("BASS_NN_ABLATE", "none"))
    if key not in _CACHE:
        _CACHE[key] = _build(float(g1), iters)
    return _CACHE[key]


def _fallback_numpy(x, Wi, Wh, Wo, gates, l1, l2):
    """Direct fp32 replication of the reference for degenerate inputs."""
    EPS = 1e-5
    Wi_b = np.sign(Wi)
    Wh_b = np.sign(Wh)
    Wo_b = np.sign(Wo)
    Bn, Hn = x.shape[1], Wi.shape[0]
    h = np.zeros((Bn, Hn), dtype=np.float32)
    outs = []
    for t in range(x.shape[0]):
        hidden = x[t] @ Wi_b.T + gates[t] * (np.sign(h) @ Wh_b.T)
        hidden = np.clip(hidden, -1.0, 1.0)
        var = hidden.var(axis=0, ddof=1, keepdims=True)
        bottom = np.sqrt(var + EPS)
        s = 1.0 / (1.0 + np.exp(-10.0 * hidden))
        hidden = (hidden * s * l1 + hidden * (1.0 - s) * l2) / bottom
        outs.append(np.sign(hidden) @ Wo_b.T)
        h = hidden
    return np.stack(outs).astype(np.float32)


def _pack_pairs(a):
    """[LO_ROWS, 2048] -> [128, 2*LO_CH*2048] with DoubleRow k-pair layout:
    out[p, (2c+j)*2048 + col] = a[c*256 + j*128 + p, col]."""
    return np.ascontiguousarray(
        a.reshape(LO_CH, 2, 128, 2048).transpose(2, 0, 1, 3).reshape(128, -1)
    )


def _pack_rows(a):
    """[HI_ROWS, 2048] -> [128, HI_CH*2048]: out[p, c*2048+col] = a[c*128+p]."""
    return np.ascontiguousarray(
        a.reshape(HI_CH, 128, 2048).transpose(1, 0, 2).reshape(128, -1)
    )


def _prep_weights(Wi, Wo, gates):
    """Shared (replicated) weight arrays."""
    Wi_b = np.sign(Wi).astype(np.float32)      # [H, IN]
    Wo_b = np.sign(Wo).astype(np.float32)      # [OUT, H]
    wT = np.ascontiguousarray(Wi_b.T)          # [IN, H]

    whi = _pack_rows(wT[:HI_ROWS].astype(np.float16))

    wlo_f = np.zeros((LO_ROWS, H), dtype=np.float32)
    wlo_f[:NLO_COV] = wT[:NLO_COV] * 2.0**-9
    wlo_f[NLO_COV : NLO_COV + LX] = wT[HI_ROWS:IN]
    wlo_f[NLO_COV + LX : NLO_COV + 2 * LX] = wT[HI_ROWS:IN] * 2.0**-4
    wlo_f[NLO_COV + 2 * LX : NLO_COV + 3 * LX] = wT[HI_ROWS:IN] * 2.0**-8
    wlo_f[NLO_COV + 3 * LX] = -1.0             # g row (u' = x@W - g_t)
    wlo = _pack_pairs(_e4m3(wlo_f))

    wo = np.ascontiguousarray(
        Wo_b.T.reshape(NHT, 128, OUT).transpose(1, 0, 2).reshape(128, -1)
    ).astype(F8)

    gamma = np.empty(T, dtype=np.float32)
    gamma[: T - 1] = gates[1:]
    gamma[T - 1] = 1.0
    dd = (-2.0 * gamma).astype(np.float32)
    d1 = np.tile(np.tile(dd, GB)[None, :], (128, 1)).astype(np.float32)

    colsum = Wo_b.sum(axis=1)
    return whi, wlo, wo, d1, gamma, colsum


def _prep_in_maps(x, Wi, Wo, gates):
    """Build the 8 per-core input maps."""
    whi, wlo, wo, d1, gamma, colsum = _prep_weights(Wi, Wo, gates)
    g_row = np.tile(gates.astype(np.float32), BS)   # [2048], col = b*T + t

    in_maps = []
    for c in range(NCORES):
        xs = x[:, c * BS : (c + 1) * BS, :]          # [T, BS, IN]
        xa = np.ascontiguousarray(xs.transpose(2, 1, 0)).reshape(IN, BS * T)
        xhi16 = xa.astype(np.float16)
        xhi = _pack_rows(xhi16[:HI_ROWS])

        lo = np.zeros((LO_ROWS, BS * T), dtype=np.float32)
        resid = xa[:NLO_COV] - xhi16[:NLO_COV].astype(np.float32)
        lo[:NLO_COV] = resid * 512.0                 # scale 2^9
        xt = xa[HI_ROWS:IN]                          # 16 leftover rows
        a8 = _e4m3(xt)
        af = a8.astype(np.float32)
        r1 = xt - af
        b8 = _e4m3(r1 * 16.0)
        r2 = r1 - b8.astype(np.float32) / 16.0
        c8 = _e4m3(r2 * 256.0)
        lo[NLO_COV : NLO_COV + LX] = af
        lo[NLO_COV + LX : NLO_COV + 2 * LX] = b8.astype(np.float32)
        lo[NLO_COV + 2 * LX : NLO_COV + 3 * LX] = c8.astype(np.float32)
        lo[NLO_COV + 3 * LX] = g_row
        xlo = _pack_pairs(_e4m3(lo))

        in_maps.append(
            {"xhi": xhi, "xlo": xlo, "whi": whi.copy(), "wlo": wlo.copy(),
             "wo": wo.copy(), "d1": d1.copy()}
        )
    return in_maps, gamma, colsum


def kernel(x, Wi, Wh, Wo, gates, l1, l2):
    x = np.asarray(x, dtype=np.float32)
    Wi = np.asarray(Wi, dtype=np.float32)
    Wh = np.asarray(Wh, dtype=np.float32)
    Wo = np.asarray(Wo, dtype=np.float32)
    gates = np.asarray(gates, dtype=np.float32)
    l1 = np.asarray(l1, dtype=np.float32)
    l2 = np.asarray(l2, dtype=np.float32)

    gamma_chk = np.empty(T, dtype=np.float32)
    if gates.shape == (T,):
        gamma_chk[: T - 1] = gates[1:]
        gamma_chk[T - 1] = 1.0
    fast = (
        x.shape == (T, B, IN)
        and gates.shape == (T,)
        and np.all(l1 > 0)
        and np.all(l2 > 0)
        and np.array_equal(np.sign(Wh), np.eye(H, dtype=np.float32))
        and np.all(gates[1:] != 0)
        and np.all(gates >= 0)            # 1-instr chain patch needs thr <= 0
        and np.all(np.abs(gates) < 1e6)
        and _e4m3_exact(gates)            # g row rides the fp8 pass
        and _e4m3_exact(-2.0 * gamma_chk)  # scan output downcast to e4m3
    )
    if not fast:
        return _fallback_numpy(x, Wi, Wh, Wo, gates, l1, l2)

    from concourse.bass_utils import run_bass_kernel_spmd

    nc = _get_module(float(gates[0]), 1)
    in_maps, gamma, colsum = _prep_in_maps(x, Wi, Wo, gates)
    res = run_bass_kernel_spmd(nc, in_maps, core_ids=list(range(NCORES)))

    out = np.empty((T, B, OUT), dtype=np.float32)
    inv_gamma = (1.0 / gamma).astype(np.float32)
    for c in range(NCORES):
        ot = res.results[c]["outt"].reshape(OUT, BS, T)
        # out[t, b, o] = -ot[o, b, t]/gamma[t] - colsum[o]
        out[:, c * BS : (c + 1) * BS, :] = (
            -ot.transpose(2, 1, 0) * inv_gamma[:, None, None]
            - colsum[None, None, :]
        )
    return out


# revision 16
# speedup vs baseline: 1.6995x; 1.0260x over previous
"""Trainium2 Bass kernel for nn_BinarizedRNN (v3).

Math: the reference's output is out[t] = sign(hidden_t) @ sign(Wo).T where
hidden feeds the next step only through sign(hidden_t).  With l1,l2 > 0 the
SignSensitiveBatchNorm factor (s*l1 + (1-s)*l2)/sqrt(var+eps) is strictly
positive, so it never changes any sign; with sign(Wh) == I the recurrent
matmul is the identity.  The whole net collapses to

    q_t = (u'_t >= p_{t-1}),  p_t = q_t * (-2*g_{t+1}),   (elementwise)
    u'_t = x_t @ sign(Wi).T - g_t                         (one big matmul)
    out_t = (2*q_t - 1) @ sign(Wo).T

v3 structure (vs the v2 baseline's bf16 hi/lo + reset-column chains):
  - chains are exactly T=64 columns (no reset columns): matmuls are clean
    N=512 (8 chains/group).  Cross-chain state leak inside the packed
    tensor_tensor_scan is neutralized by a DVE patch that rewrites each
    chain's first PSUM column to +-1e9 according to (u'_0 >= -g_1); the
    scan's compile-time `initial` covers chain 0.
  - mm1 = 6 fp16 chunks (x rows 0..767, 11-bit significand) + 4 fp8e4
    DoubleRow chunks at 2x column rate: lo-residuals (x - fp16(x)) scaled
    2^9 against weights +-2^-9, the leftover x rows 768..783 as a 3-way
    fp8 split (scales 1 / 2^-4 / 2^-8), plus the g row.
  - mm2 (output matmul) in fp8 DoubleRow over the fp8 scan output
    (values {0, -2*gamma_t}, exact in e4m3 when gamma is).
  - weight DMAs hoisted out of the timing loop; 2x-unrolled loop body with
    double-buffered x tiles so input DMA pipelines across iterations.
"""
import os
import numpy as np
import ml_dtypes

T, B, IN, H, OUT = 64, 256, 784, 2048, 256
NCORES = 8
BS = B // NCORES          # 32 chains per core
NG = 4                    # chain groups per core
GB = BS // NG             # 8 chains per group
NCOL = GB * T             # 512 columns per group
NHT = H // 128            # 16
HHT = NHT // 2            # ht tiles per scan half
SCN = HHT * NCOL          # 4096 columns per scan
NO = OUT // 128           # 2
HI_CH = 6                 # fp16 K-chunks (rows 0..767)
HI_ROWS = HI_CH * 128     # 768
LO_CH = int(os.environ.get("BASS_NN_LOCH", "3"))  # fp8 DoubleRow K-chunks
LO_ROWS = LO_CH * 256     # fp8 row slots
LX = IN - HI_ROWS         # 16 leftover x rows
# fp16 rows that get an fp8 lo-residual row (the rest stay fp16-only;
# each skipped row costs ~2^-11 relative error instead of 2^-15)
NLO_COV = min(HI_ROWS, LO_ROWS - 3 * LX - 1)
BIGP = 1e9

F8 = ml_dtypes.float8_e4m3

_CACHE = {}


def _e4m3(a):
    return np.asarray(a, dtype=np.float32).astype(F8)


def _e4m3_exact(a):
    a = np.asarray(a, dtype=np.float32)
    return bool(np.all(_e4m3(a).astype(np.float32) == a))


def _build(g1: float, iters: int = 1):
    import contextlib
    import concourse.bacc as bacc
    import concourse.mybir as mybir
    import concourse.tile as tile

    f32 = mybir.dt.float32
    f16 = mybir.dt.float16
    f8 = mybir.dt.float8e4
    DR = mybir.MatmulPerfMode.DoubleRow
    ALU = mybir.AluOpType

    nc = bacc.Bacc(
        "TRN2", target_bir_lowering=False, debug=False, num_devices=NCORES
    )

    bf16 = mybir.dt.bfloat16

    xhi_d = nc.dram_tensor("xhi", [128, HI_CH * 2048], f16, kind="ExternalInput")
    xlo_d = nc.dram_tensor("xlo", [128, 2 * LO_CH * 2048], f8, kind="ExternalInput")
    whi_d = nc.dram_tensor("whi", [128, HI_CH * 2048], f16, kind="ExternalInput")
    wlo_d = nc.dram_tensor("wlo", [128, 2 * LO_CH * 2048], f8, kind="ExternalInput")
    wo_d = nc.dram_tensor("wo", [128, NHT * OUT], f8, kind="ExternalInput")
    d1_d = nc.dram_tensor("d1", [128, SCN], bf16, kind="ExternalInput")
    outt_d = nc.dram_tensor("outt", [OUT, BS * T], f32, kind="ExternalOutput")

    ablate = os.environ.get("BASS_NN_ABLATE", "none")

    with tile.TileContext(nc) as tc:
        with (
            tc.tile_pool(name="wpool", bufs=1) as wpool,
            tc.tile_pool(name="xpool", bufs=2) as xpool,
            tc.tile_pool(name="upool", bufs=3) as upool,
            tc.tile_pool(name="ppool", bufs=2) as ppool,
            tc.tile_pool(name="stage", bufs=4) as stage,
            tc.tile_pool(name="ps1", bufs=4, space="PSUM") as ps1,
            tc.tile_pool(name="ps2", bufs=2, space="PSUM") as ps2,
        ):
            # ---- resident weights (loaded once per invocation) ----
            whi_t = wpool.tile([128, HI_CH, 2048], f16, tag="whi")
            nc.sync.dma_start(whi_t[:].rearrange("p a b -> p (a b)"), whi_d[:])
            wlo_t = wpool.tile([128, 2 * LO_CH, 2048], f8, tag="wlo")
            nc.sync.dma_start(wlo_t[:].rearrange("p a b -> p (a b)"), wlo_d[:])
            wo_t = wpool.tile([128, NHT, OUT], f8, tag="wo")
            nc.sync.dma_start(wo_t[:].rearrange("p a b -> p (a b)"), wo_d[:])
            d1_t = wpool.tile([128, SCN], bf16, tag="d1")
            nc.sync.dma_start(d1_t[:], d1_d[:])

            unroll = 1 if iters == 1 else 2
            assert iters == 1 or iters % unroll == 0

            def emit_mm2(p_t, g, sub):
                for o in range(NO):
                    po = ps2.tile([128, NCOL], f32, tag="mm2",
                                  name=f"po_{sub}_{g}_{o}")
                    for hp in range(NHT // 2):
                        nc.tensor.matmul(
                            po[:],
                            wo_t[:, 2 * hp : 2 * hp + 2, o * 128 : (o + 1) * 128],
                            p_t[:, 2 * hp : 2 * hp + 2, :],
                            start=(hp == 0),
                            stop=(hp == NHT // 2 - 1),
                            perf_mode=DR,
                        )
                    st = stage.tile([128, NCOL], f32, tag="st",
                                    name=f"st_{sub}_{g}_{o}")
                    nc.scalar.copy(st[:], po[:])
                    nc.sync.dma_start(
                        outt_d[o * 128 : (o + 1) * 128,
                               g * NCOL : (g + 1) * NCOL],
                        st[:],
                    )

            def emit_iter(sub, pending):
                xhi_t = xpool.tile([128, HI_CH, 2048], f16, tag="xhi",
                                   name=f"xhi_{sub}")
                nc.sync.dma_start(
                    xhi_t[:].rearrange("p a b -> p (a b)"), xhi_d[:]
                )
                xlo_t = xpool.tile([128, 2 * LO_CH, 2048], f8, tag="xlo",
                                   name=f"xlo_{sub}")
                nc.sync.dma_start(
                    xlo_t[:].rearrange("p a b -> p (a b)"), xlo_d[:]
                )

                # mm1 chunk schedule: interleave fp16/DoubleRow chunks so
                # every 213ns DR weight-load hides under an fp16 matmul
                chunks = []
                nlo = LO_CH if ablate != "nolo" else 0
                li = 0
                for ci in range(HI_CH):
                    chunks.append(("hi", ci))
                    if li < nlo:
                        chunks.append(("lo", li))
                        li += 1
                while li < nlo:
                    chunks.append(("lo", li))
                    li += 1
                n_mm = len(chunks)

                p_prev = None
                for g in range(NG):
                    p_t = ppool.tile([128, NHT, NCOL], f8, tag="p",
                                     name=f"p_{sub}_{g}")
                    u_sb = None
                    cs = g * NCOL
                    for ht in range(NHT):
                        h, hi = divmod(ht, HHT)
                        if hi == 0:
                            u_sb = upool.tile([128, HHT, NCOL], f32, tag="u",
                                              name=f"u_{sub}_{g}_{h}")
                        ps = ps1.tile([128, NCOL], f32, tag="mm1",
                                      name=f"ps_{sub}_{g}_{ht}")
                        for i, (kind, ci) in enumerate(chunks):
                            if kind == "hi":
                                nc.tensor.matmul(
                                    ps[:],
                                    whi_t[:, ci, ht * 128 : (ht + 1) * 128],
                                    xhi_t[:, ci, cs : cs + NCOL],
                                    start=(i == 0),
                                    stop=(i == n_mm - 1),
                                )
                            else:
                                nc.tensor.matmul(
                                    ps[:],
                                    wlo_t[:, 2 * ci : 2 * ci + 2,
                                          ht * 128 : (ht + 1) * 128],
                                    xlo_t[:, 2 * ci : 2 * ci + 2, cs : cs + NCOL],
                                    start=(i == 0),
                                    stop=(i == n_mm - 1),
                                    perf_mode=DR,
                                )
                        # ACT drains PSUM fast so the PE never waits on the
                        # (slow) DVE scan
                        nc.scalar.copy(u_sb[:, hi, :], ps[:])
                        if hi == HHT - 1:
                            # chain-boundary patch: first col of each chain ->
                            # {0, -2e9}: decision-equivalent to u'_0 vs -g1
                            # because all scan thresholds are <= 0 (gates >= 0)
                            if ablate != "nopatch":
                                v = u_sb[:].rearrange(
                                    "p a (c t) -> p (a c) t", t=T
                                )[:, :, 0]
                                nc.vector.tensor_scalar(
                                    v, v, -g1, -2 * BIGP, ALU.is_lt, ALU.mult
                                )
                            # one long scan per half-group (8 ht-tiles);
                            # chain/tile boundaries are all patched columns
                            if ablate != "noscan":
                                nc.vector.tensor_tensor_scan(
                                    p_t[:, h * HHT : (h + 1) * HHT, :].rearrange(
                                        "p a b -> p (a b)"
                                    ),
                                    u_sb[:].rearrange("p a b -> p (a b)"),
                                    d1_t[:],
                                    -g1,
                                    ALU.is_ge,
                                    ALU.mult,
                                )
                        # interleave deferred output-matmuls behind the first
                        # mm1 tiles so the PE never stalls on a scan tail
                        if ht == 1 and pending and ablate != "nomm2":
                            emit_mm2(*pending.pop(0))
                    if p_prev is not None and ablate != "nomm2":
                        emit_mm2(p_prev, g - 1, sub)
                    p_prev = p_t
                return [(p_prev, NG - 1, sub)]

            with (
                tc.For_i(0, iters // unroll, 1)
                if iters > 1
                else contextlib.nullcontext()
            ):
                pending = []
                for sub in range(unroll):
                    pending = emit_iter(sub, pending)
                for args in pending:
                    if ablate != "nomm2":
                        emit_mm2(*args)

    nc.compile()
    return nc


def _get_module(g1: float, iters: int = 1):
    key = (float(g1), iters, os.environ.get("BASS_NN_ABLATE", "none"))
    if key not in _CACHE:
        _CACHE[key] = _build(float(g1), iters)
    return _CACHE[key]


def _fallback_numpy(x, Wi, Wh, Wo, gates, l1, l2):
    """Direct fp32 replication of the reference for degenerate inputs."""
    EPS = 1e-5
    Wi_b = np.sign(Wi)
    Wh_b = np.sign(Wh)
    Wo_b = np.sign(Wo)
    Bn, Hn = x.shape[1], Wi.shape[0]
    h = np.zeros((Bn, Hn), dtype=np.float32)
    outs = []
    for t in range(x.shape[0]):
        hidden = x[t] @ Wi_b.T + gates[t] * (np.sign(h) @ Wh_b.T)
        hidden = np.clip(hidden, -1.0, 1.0)
        var = hidden.var(axis=0, ddof=1, keepdims=True)
        bottom = np.sqrt(var + EPS)
        s = 1.0 / (1.0 + np.exp(-10.0 * hidden))
        hidden = (hidden * s * l1 + hidden * (1.0 - s) * l2) / bottom
        outs.append(np.sign(hidden) @ Wo_b.T)
        h = hidden
    return np.stack(outs).astype(np.float32)


def _pack_pairs(a):
    """[LO_ROWS, 2048] -> [128, 2*LO_CH*2048] with DoubleRow k-pair layout:
    out[p, (2c+j)*2048 + col] = a[c*256 + j*128 + p, col]."""
    return np.ascontiguousarray(
        a.reshape(LO_CH, 2, 128, 2048).transpose(2, 0, 1, 3).reshape(128, -1)
    )


def _pack_rows(a):
    """[HI_ROWS, 2048] -> [128, HI_CH*2048]: out[p, c*2048+col] = a[c*128+p]."""
    return np.ascontiguousarray(
        a.reshape(HI_CH, 128, 2048).transpose(1, 0, 2).reshape(128, -1)
    )


def _prep_weights(Wi, Wo, gates):
    """Shared (replicated) weight arrays."""
    Wi_b = np.sign(Wi).astype(np.float32)      # [H, IN]
    Wo_b = np.sign(Wo).astype(np.float32)      # [OUT, H]
    wT = np.ascontiguousarray(Wi_b.T)          # [IN, H]

    whi = _pack_rows(wT[:HI_ROWS].astype(np.float16))

    wlo_f = np.zeros((LO_ROWS, H), dtype=np.float32)
    wlo_f[:NLO_COV] = wT[:NLO_COV] * 2.0**-9
    wlo_f[NLO_COV : NLO_COV + LX] = wT[HI_ROWS:IN]
    wlo_f[NLO_COV + LX : NLO_COV + 2 * LX] = wT[HI_ROWS:IN] * 2.0**-4
    wlo_f[NLO_COV + 2 * LX : NLO_COV + 3 * LX] = wT[HI_ROWS:IN] * 2.0**-8
    wlo_f[NLO_COV + 3 * LX] = -1.0             # g row (u' = x@W - g_t)
    wlo = _pack_pairs(_e4m3(wlo_f))

    wo = np.ascontiguousarray(
        Wo_b.T.reshape(NHT, 128, OUT).transpose(1, 0, 2).reshape(128, -1)
    ).astype(F8)

    gamma = np.empty(T, dtype=np.float32)
    gamma[: T - 1] = gates[1:]
    gamma[T - 1] = 1.0
    dd = (-2.0 * gamma).astype(np.float32)
    d1 = np.tile(np.tile(dd, GB * HHT)[None, :], (128, 1)).astype(
        ml_dtypes.bfloat16
    )

    colsum = Wo_b.sum(axis=1)
    return whi, wlo, wo, d1, gamma, colsum


def _prep_in_maps(x, Wi, Wo, gates):
    """Build the 8 per-core input maps."""
    whi, wlo, wo, d1, gamma, colsum = _prep_weights(Wi, Wo, gates)
    g_row = np.tile(gates.astype(np.float32), BS)   # [2048], col = b*T + t

    in_maps = []
    for c in range(NCORES):
        xs = x[:, c * BS : (c + 1) * BS, :]          # [T, BS, IN]
        xa = np.ascontiguousarray(xs.transpose(2, 1, 0)).reshape(IN, BS * T)
        xhi16 = xa.astype(np.float16)
        xhi = _pack_rows(xhi16[:HI_ROWS])

        lo = np.zeros((LO_ROWS, BS * T), dtype=np.float32)
        resid = xa[:NLO_COV] - xhi16[:NLO_COV].astype(np.float32)
        lo[:NLO_COV] = resid * 512.0                 # scale 2^9
        xt = xa[HI_ROWS:IN]                          # 16 leftover rows
        a8 = _e4m3(xt)
        af = a8.astype(np.float32)
        r1 = xt - af
        b8 = _e4m3(r1 * 16.0)
        r2 = r1 - b8.astype(np.float32) / 16.0
        c8 = _e4m3(r2 * 256.0)
        lo[NLO_COV : NLO_COV + LX] = af
        lo[NLO_COV + LX : NLO_COV + 2 * LX] = b8.astype(np.float32)
        lo[NLO_COV + 2 * LX : NLO_COV + 3 * LX] = c8.astype(np.float32)
        lo[NLO_COV + 3 * LX] = g_row
        xlo = _pack_pairs(_e4m3(lo))

        in_maps.append(
            {"xhi": xhi, "xlo": xlo, "whi": whi.copy(), "wlo": wlo.copy(),
             "wo": wo.copy(), "d1": d1.copy()}
        )
    return in_maps, gamma, colsum


def kernel(x, Wi, Wh, Wo, gates, l1, l2):
    x = np.asarray(x, dtype=np.float32)
    Wi = np.asarray(Wi, dtype=np.float32)
    Wh = np.asarray(Wh, dtype=np.float32)
    Wo = np.asarray(Wo, dtype=np.float32)
    gates = np.asarray(gates, dtype=np.float32)
    l1 = np.asarray(l1, dtype=np.float32)
    l2 = np.asarray(l2, dtype=np.float32)

    gamma_chk = np.empty(T, dtype=np.float32)
    if gates.shape == (T,):
        gamma_chk[: T - 1] = gates[1:]
        gamma_chk[T - 1] = 1.0
    fast = (
        x.shape == (T, B, IN)
        and gates.shape == (T,)
        and np.all(l1 > 0)
        and np.all(l2 > 0)
        and np.array_equal(np.sign(Wh), np.eye(H, dtype=np.float32))
        and np.all(gates[1:] != 0)
        and np.all(gates >= 0)            # 1-instr chain patch needs thr <= 0
        and np.all(np.abs(gates) < 1e6)
        and _e4m3_exact(gates)            # g row rides the fp8 pass
        and _e4m3_exact(-2.0 * gamma_chk)  # scan output downcast to e4m3
    )
    if not fast:
        return _fallback_numpy(x, Wi, Wh, Wo, gates, l1, l2)

    from concourse.bass_utils import run_bass_kernel_spmd

    nc = _get_module(float(gates[0]), 1)
    in_maps, gamma, colsum = _prep_in_maps(x, Wi, Wo, gates)
    res = run_bass_kernel_spmd(nc, in_maps, core_ids=list(range(NCORES)))

    out = np.empty((T, B, OUT), dtype=np.float32)
    inv_gamma = (1.0 / gamma).astype(np.float32)
    for c in range(NCORES):
        ot = res.results[c]["outt"].reshape(OUT, BS, T)
        # out[t, b, o] = -ot[o, b, t]/gamma[t] - colsum[o]
        out[:, c * BS : (c + 1) * BS, :] = (
            -ot.transpose(2, 1, 0) * inv_gamma[:, None, None]
            - colsum[None, None, :]
        )
    return out
